# revision 1
# baseline (speedup 1.0000x reference)
"""Trainium2 Bass kernel for nn_AdapterLayer (MoE routing adapter).

Strategy (8 NeuronCores, data-parallel over batch B=8, one batch per core):
  pass1: high-pass 3x3 depthwise conv computed ON THE PE as 3 PSUM-accumulated
         matmuls (tridiagonal h-mixing weights x 3 shifted-w access patterns
         on a [h, d, w_padded] layout), GELU on ACT, spatial sums via DVE
         w-reduce + PE ones-matmul h-reduce.
  routing: tiny MLP + softmax + top-2 done with small matmuls / DVE vector ops;
         the top-2 expert *selection* is done with 0/1 selection matrices built
         on-device and applied as matmuls against the stacked expert weights
         (no data-dependent control flow, graph stays static SPMD).
  pass2: only the 2 selected experts are computed:
         y = (P0sel x) * silu(P1sel shared);  out = (g.WoP2)sel y + gs.Wo x
         (proj_out folded into the expert combine on the host).
  All big matmuls run in bf16 (full PE rate, half DMA; fp32 PSUM accum).
"""

import sys

if "/opt/trn_rl_repo" not in sys.path:
    sys.path.insert(0, "/opt/trn_rl_repo")

import numpy as np
from contextlib import ExitStack

import concourse.bass as bass
import concourse.tile as tile
from concourse import bacc, mybir
from concourse.bass_utils import run_bass_kernel_spmd

DIM = 256
RANK = 128
E = 4
B = 8
H = 128
W = 128
HW = H * W
WPAD = W + 2
ER = E * RANK          # 512
HID = 2 * DIM          # 512
F32 = mybir.dt.float32
BF16 = mybir.dt.bfloat16
AF = mybir.ActivationFunctionType
ALU = mybir.AluOpType
AX = mybir.AxisListType

NTILE = 64             # pass1 tiles: one per 4-channel group, full w
NCHUNK = 8             # pass1 xh DMA chunks (32 channels each)
P2TILE = 32            # pass2 tiles of 512 columns of hw
P2Q = 8                # pass2 DMA chunks (2048 cols each)
P2PER = P2TILE // P2Q  # tiles per DMA chunk


def build_nc(sim_safe=False, reps=1):
    # sim_safe: CoreSim lacks Gelu/Silu tables; substitute implemented funcs
    # (Identity/Sigmoid) so the full dataflow can be validated in simulation.
    # reps: emit the kernel body N times back-to-back (used by test.py to
    # measure per-iteration hardware time by differencing NEFF wall times).
    global _AF_GELU, _AF_SILU
    _AF_GELU = AF.Identity if sim_safe else AF.Gelu
    _AF_SILU = AF.Sigmoid if sim_safe else AF.Silu
    nc = bacc.Bacc("TRN2", target_bir_lowering=False, debug=False)

    def din(name, shape, dt=F32):
        return nc.dram_tensor(name, shape, dt, kind="ExternalInput").ap()

    xh_d = din("xh", [H, DIM * WPAD], BF16)  # x as [h, (d, w_pad)]
    xd_d = din("xd", [DIM, HW], BF16)        # x as [d, h*w]
    sh_d = din("sh", [DIM, HW], BF16)        # shared as [d, h*w]
    wa_d = din("wa", [ER, DIM], BF16)        # p0 stacked [er, d]
    wb_d = din("wb", [ER, DIM], BF16)        # p1 stacked [er, d]
    wo_d = din("wo", [ER, DIM], BF16)        # (Wo @ p2) stacked [er, o]
    wx_d = din("wx", [DIM, DIM], BF16)       # Wo.T [d, o]
    w1_d = din("w1", [DIM, HID])            # mlp_w1.T / HW  [d, hid]
    b1_d = din("b1v", [HID, 1])
    w2_d = din("w2", [HID, DIM])            # mlp_w2.T [hid, d]
    b2_d = din("b2v", [DIM, 1])
    gw_d = din("gw", [DIM, E])              # gate_w.T / HW
    fgw_d = din("fgw", [DIM, E])            # freq_gate_w.T
    thw_d = din("thw", [3 * 128, 128], BF16)  # conv tap lhsT matrices
    onec_d = din("onec", [128, 1])          # ones column
    oner_d = din("oner", [1, 128])          # ones row
    iden_d = din("iden", [128, 128])        # identity

    out_d = nc.dram_tensor("out", [DIM, HW], F32, kind="ExternalOutput").ap()

    env = locals()
    with tile.TileContext(nc) as tc:
        for _ in range(reps):
            _body(tc, env)
    nc.compile()
    return nc


def _body(tc, t):
    nc = tc.nc
    xh_d, xd_d, sh_d = t["xh_d"], t["xd_d"], t["sh_d"]
    out_d = t["out_d"]

    with ExitStack() as ctx:
        wk = ctx.enter_context(tc.tile_pool(name="wk", bufs=1))

        def load_tiled(tag, dram_ap, ntiles, m, dt=F32):
            """DRAM [ntiles*128, m] -> SBUF [128, ntiles*m] (tile a at cols a*m)."""
            s = wk.tile([128, ntiles * m], dt, tag=tag)
            nc.sync.dma_start(
                s[:].rearrange("p (a m) -> p a m", a=ntiles),
                dram_ap.rearrange("(a p) m -> p a m", p=128),
            )
            return s

        # ---- persistent weight tiles (loaded once) ----
        thw_s = load_tiled("thw", t["thw_d"][:], 3, 128, BF16)
        onec_s = wk.tile([128, 1], F32, tag="onec")
        nc.sync.dma_start(onec_s[:], t["onec_d"][:])
        oner_s = wk.tile([1, 128], F32, tag="oner")
        nc.sync.dma_start(oner_s[:], t["oner_d"][:])
        iden_s = wk.tile([128, 128], F32, tag="iden")
        nc.sync.dma_start(iden_s[:], t["iden_d"][:])
        wa_s = load_tiled("wa", t["wa_d"][:], 4, DIM, BF16)
        wb_s = load_tiled("wb", t["wb_d"][:], 4, DIM, BF16)
        wo_s = load_tiled("wo", t["wo_d"][:], 4, DIM, BF16)
        wx_s = load_tiled("wx", t["wx_d"][:], 2, DIM, BF16)
        w1_s = load_tiled("w1", t["w1_d"][:], 2, HID)
        w2_s = load_tiled("w2", t["w2_d"][:], 4, DIM)
        b1_s = load_tiled("b1", t["b1_d"][:], 4, 1)
        b2_s = load_tiled("b2", t["b2_d"][:], 2, 1)
        gw_s = load_tiled("gw", t["gw_d"][:], 2, E)
        fgw_s = load_tiled("fgw", t["fgw_d"][:], 2, E)

        # accumulators for the two spatial reductions
        gacc = wk.tile([128, DIM], F32, tag="gacc")   # [h', d] sums over w of gelu
        # per-(dc,chunk) partial x sums from ACT/DVE accum_out (pooled path)
        NXQ = 4
        xpp = wk.tile([128, 2 * NXQ], F32, tag="xpp")
        xscr = wk.tile([128, HW // NXQ], BF16, tag="xscr")  # accum-op scratch

        # x in [d, hw] layout stays RESIDENT in SBUF: used here for the pooled
        # sums (accum_out reduces along free = exactly sum over (h,w) per d)
        # and again in pass 2 as the matmul moving operand -- one DMA total.
        xdres = wk.tile([128, 2 * HW], BF16, tag="xdres")
        CQ1 = HW // NXQ
        for q in range(NXQ):
            for dc in range(2):
                sl = xdres[:, dc * HW + q * CQ1: dc * HW + (q + 1) * CQ1]
                nc.scalar.dma_start(sl, xd_d[dc * 128:(dc + 1) * 128,
                                             q * CQ1:(q + 1) * CQ1])
                acc = xpp[:, dc * NXQ + q: dc * NXQ + q + 1]
                if dc == 0:
                    nc.scalar.activation(xscr[:], sl, AF.Copy, accum_out=acc)
                else:
                    nc.vector.tensor_scalar(out=xscr[:], in0=sl, scalar1=1.0,
                                            scalar2=0.0, op0=ALU.mult,
                                            op1=ALU.add, accum_out=acc)

        with (
            tc.tile_pool(name="xhp", bufs=2) as xh_pool,
            tc.tile_pool(name="hp_ps", bufs=4, space="PSUM") as hp_ps_pool,
            tc.tile_pool(name="gelu", bufs=4) as gelu_pool,
        ):
            CW = 32 * WPAD  # columns per xh chunk
            for c in range(NCHUNK):
                xh_t = xh_pool.tile([128, CW], BF16, tag="xh")
                nc.sync.dma_start(xh_t[:], xh_d[:, c * CW:(c + 1) * CW])
                xh3 = xh_t[:].rearrange("p (d w) -> p d w", w=WPAD)
                for j in range(8):          # 8 groups of 4 channels per chunk
                    g = c * 8 + j
                    hp = hp_ps_pool.tile([128, 512], F32, tag="hp", space="PSUM")
                    for dw in range(3):
                        nc.tensor.matmul(
                            hp[:],
                            (thw_s[:, dw * 128:(dw + 1) * 128]),
                            (xh3[:, j * 4:(j + 1) * 4, dw:dw + W]),
                            start=(dw == 0), stop=(dw == 2),
                        )
                    gelu_t = gelu_pool.tile([128, 512], F32, tag="gelu")
                    nc.scalar.activation(gelu_t[:], hp[:], _AF_GELU)
                    nc.vector.tensor_reduce(
                        out=gacc[:, g * 4:(g + 1) * 4],
                        in_=gelu_t[:].rearrange("p (d w) -> p d w", w=W),
                        axis=AX.X, op=ALU.add,
                    )

        # ======================= routing (tiny) =======================
        pooled_s = wk.tile([128, 2], F32, tag="pooled")  # x means (sum; /HW folded in gw)
        gmean_s = wk.tile([128, 2], F32, tag="gmean")    # gelu sums (/HW folded in w1)
        hid_s = wk.tile([128, 4], F32, tag="hid")
        freq_s = wk.tile([128, 2], F32, tag="freq")
        sv = wk.tile([1, 40], F32, tag="sv")             # scratch vector lane
        bc_s = wk.tile([128, 17], F32, tag="bc")
        su_s = wk.tile([128, 4 * 256], BF16, tag="su")    # S_unit er-tiles
        sg_s = wk.tile([128, 4 * 256], BF16, tag="sg")    # S_gated er-tiles
        a_lh = wk.tile([128, 2 * 256], BF16, tag="a_lh")  # selected p0 lhsT [d, rsel]
        b_lh = wk.tile([128, 2 * 256], BF16, tag="b_lh")
        o_lh = wk.tile([128, 2 * 256], BF16, tag="o_lh")  # selected g*WoP2 lhsT [rsel, o]
        x_lh = wk.tile([128, 2 * 256], BF16, tag="x_lh")  # gs * Wo.T tiles [d, o]

        with tc.tile_pool(name="sm_ps", bufs=2, space="PSUM") as sp:
            # pooled from ACT accum partials: sum the 8 chunk-columns per dc
            nc.vector.tensor_reduce(
                out=pooled_s[:],
                in_=xpp[:].rearrange("p (dc q) -> p dc q", q=NXQ),
                axis=AX.X, op=ALU.add,
            )
            # gelu-mean column sums over h via ones-matmul as [128,1] chunks
            for dc in range(2):
                ps2 = sp.tile([128, 1], F32, tag="sums", space="PSUM")
                nc.tensor.matmul(ps2[:], gacc[:, dc * 128:(dc + 1) * 128], onec_s[:],
                                 start=True, stop=True)
                nc.scalar.copy(gmean_s[:, dc:dc + 1], ps2[:])

            # MLP: hidden = gelu(gmean @ w1T + b1)  (4 chunks of 128)
            for mh in range(4):
                ps = sp.tile([128, 1], F32, tag="mlp", space="PSUM")
                for dc in range(2):
                    nc.tensor.matmul(
                        ps[:],
                        w1_s[:, dc * HID + mh * 128: dc * HID + (mh + 1) * 128],
                        gmean_s[:, dc:dc + 1],
                        start=(dc == 0), stop=(dc == 1),
                    )
                nc.scalar.activation(hid_s[:, mh:mh + 1], ps[:], _AF_GELU,
                                     bias=b1_s[:, mh:mh + 1])
            # freq = hidden @ w2T + b2 (2 chunks of 128)
            for dc in range(2):
                ps = sp.tile([128, 1], F32, tag="mlp", space="PSUM")
                for kh in range(4):
                    nc.tensor.matmul(
                        ps[:],
                        w2_s[:, kh * DIM + dc * 128: kh * DIM + (dc + 1) * 128],
                        hid_s[:, kh:kh + 1],
                        start=(kh == 0), stop=(kh == 3),
                    )
                nc.scalar.activation(freq_s[:, dc:dc + 1], ps[:], AF.Identity,
                                     bias=b2_s[:, dc:dc + 1])
            # logits = pooled @ gw + freq @ fgw  -> [1, 4]
            lg_ps = sp.tile([1, E], F32, tag="lg", space="PSUM")
            for dc in range(2):
                nc.tensor.matmul(lg_ps[:], pooled_s[:, dc:dc + 1],
                                 gw_s[:, dc * E:(dc + 1) * E],
                                 start=(dc == 0), stop=False)
            for dc in range(2):
                nc.tensor.matmul(lg_ps[:], freq_s[:, dc:dc + 1],
                                 fgw_s[:, dc * E:(dc + 1) * E],
                                 start=False, stop=(dc == 1))
            lg = sv[:, 0:4]
            nc.scalar.copy(lg, lg_ps[:])

            # softmax over 4
            mx = sv[:, 4:5]
            nc.vector.tensor_reduce(out=mx, in_=lg, axis=AX.X, op=ALU.max)
            shf = sv[:, 5:9]
            nc.vector.tensor_scalar(out=shf, in0=lg, scalar1=mx, scalar2=None,
                                    op0=ALU.subtract)
            u = sv[:, 9:13]
            nc.scalar.activation(u, shf, AF.Exp)
            z = sv[:, 13:14]
            nc.vector.tensor_reduce(out=z, in_=u, axis=AX.X, op=ALU.add)
            zr = sv[:, 38:39]
            nc.vector.reciprocal(zr, z)
            gn = sv[:, 14:18]
            nc.vector.tensor_scalar(out=gn, in0=u, scalar1=zr, scalar2=None,
                                    op0=ALU.mult)
            # top-2 masks
            m1 = sv[:, 18:19]
            nc.vector.tensor_reduce(out=m1, in_=gn, axis=AX.X, op=ALU.max)
            eq1 = sv[:, 19:23]
            nc.vector.tensor_scalar(out=eq1, in0=gn, scalar1=m1, scalar2=None,
                                    op0=ALU.is_equal)
            v2 = sv[:, 23:27]
            nc.vector.tensor_sub(v2, gn, eq1)
            m2 = sv[:, 27:28]
            nc.vector.tensor_reduce(out=m2, in_=v2, axis=AX.X, op=ALU.max)
            eq2 = sv[:, 28:32]
            nc.vector.tensor_scalar(out=eq2, in0=gn, scalar1=m2, scalar2=None,
                                    op0=ALU.is_equal)
            # bvec = [eq1(4), eq2(4), m1*eq1(4), m2*eq2(4), m1+m2(1)]
            bvec = wk.tile([1, 17], F32, tag="bvec")
            nc.vector.tensor_copy(bvec[:, 0:4], eq1)
            nc.vector.tensor_copy(bvec[:, 4:8], eq2)
            nc.vector.tensor_scalar(out=bvec[:, 8:12], in0=eq1, scalar1=m1,
                                    scalar2=None, op0=ALU.mult)
            nc.vector.tensor_scalar(out=bvec[:, 12:16], in0=eq2, scalar1=m2,
                                    scalar2=None, op0=ALU.mult)
            nc.vector.tensor_scalar(out=bvec[:, 16:17], in0=m1, scalar1=m2,
                                    scalar2=None, op0=ALU.add)

            # broadcast to all 128 partitions via K=1 matmul
            bc_ps = sp.tile([128, 17], F32, tag="bc", space="PSUM")
            nc.tensor.matmul(bc_ps[:], oner_s[:], bvec[:], start=True, stop=True)
            nc.scalar.copy(bc_s[:], bc_ps[:])

        # S matrices: per expert-tile e, slot columns scaled identities
        for e in range(E):
            nc.vector.tensor_scalar(out=su_s[:, e * 256:e * 256 + 128], in0=iden_s[:],
                                    scalar1=bc_s[:, e:e + 1], scalar2=None, op0=ALU.mult)
            nc.vector.tensor_scalar(out=su_s[:, e * 256 + 128:(e + 1) * 256], in0=iden_s[:],
                                    scalar1=bc_s[:, 4 + e:5 + e], scalar2=None, op0=ALU.mult)
            nc.vector.tensor_scalar(out=sg_s[:, e * 256:e * 256 + 128], in0=iden_s[:],
                                    scalar1=bc_s[:, 8 + e:9 + e], scalar2=None, op0=ALU.mult)
            nc.vector.tensor_scalar(out=sg_s[:, e * 256 + 128:(e + 1) * 256], in0=iden_s[:],
                                    scalar1=bc_s[:, 12 + e:13 + e], scalar2=None, op0=ALU.mult)

        # selection matmuls
        with tc.tile_pool(name="sel_ps", bufs=2, space="PSUM") as selp:
            for dc in range(2):
                ps = selp.tile([128, 256], F32, tag="sel", space="PSUM")
                for kt in range(4):
                    nc.tensor.matmul(
                        ps[:],
                        (wa_s[:, kt * DIM + dc * 128: kt * DIM + dc * 128 + 128]),
                        (su_s[:, kt * 256:(kt + 1) * 256]),
                        start=(kt == 0), stop=(kt == 3),
                    )
                nc.scalar.copy(a_lh[:, dc * 256:(dc + 1) * 256], ps[:])
                ps = selp.tile([128, 256], F32, tag="sel", space="PSUM")
                for kt in range(4):
                    nc.tensor.matmul(
                        ps[:],
                        (wb_s[:, kt * DIM + dc * 128: kt * DIM + dc * 128 + 128]),
                        (su_s[:, kt * 256:(kt + 1) * 256]),
                        start=(kt == 0), stop=(kt == 3),
                    )
                nc.scalar.copy(b_lh[:, dc * 256:(dc + 1) * 256], ps[:])
            for ms in range(2):
                ps = selp.tile([128, 256], F32, tag="sel", space="PSUM")
                for kt in range(4):
                    nc.tensor.matmul(
                        ps[:],
                        (sg_s[:, kt * 256 + ms * 128: kt * 256 + ms * 128 + 128]),
                        (wo_s[:, kt * DIM:(kt + 1) * DIM]),
                        start=(kt == 0), stop=(kt == 3),
                    )
                nc.scalar.copy(o_lh[:, ms * 256:(ms + 1) * 256], ps[:])
        # gs * Wo.T
        for dc in range(2):
            nc.vector.tensor_scalar(out=x_lh[:, dc * 256:(dc + 1) * 256],
                                    in0=wx_s[:, dc * 256:(dc + 1) * 256],
                                    scalar1=bc_s[:, 16:17], scalar2=None, op0=ALU.mult)

        # =========================== pass 2 ===========================
        CQ = HW // P2Q  # 2048 columns per DMA chunk
        with (
            tc.tile_pool(name="shp", bufs=2) as sh_pool,
            tc.tile_pool(name="pa", bufs=3, space="PSUM") as pa_pool,
            tc.tile_pool(name="pb", bufs=3, space="PSUM") as pb_pool,
            tc.tile_pool(name="po", bufs=2, space="PSUM") as po_pool,
            tc.tile_pool(name="p2sb", bufs=3) as p2sb,
            tc.tile_pool(name="osb", bufs=4) as osb_pool,
        ):
            for q in range(P2Q):
                st = []
                for dc in range(2):
                    st_t = sh_pool.tile([128, CQ], BF16, tag=f"sh{dc}")
                    nc.scalar.dma_start(st_t[:], sh_d[dc * 128:(dc + 1) * 128,
                                                      q * CQ:(q + 1) * CQ])
                    st.append(st_t)
                xt = [xdres[:, dc * HW + q * CQ: dc * HW + (q + 1) * CQ]
                      for dc in range(2)]
                for j in range(P2PER):
                    n = q * P2PER + j
                    cols = bass.ts(j, 512)
                    a_ps = []
                    y_sb = []
                    for s in range(2):
                        aps = pa_pool.tile([128, 512], F32, tag="a", space="PSUM")
                        for dc in range(2):
                            nc.tensor.matmul(
                                aps[:],
                                (a_lh[:, dc * 256 + s * 128: dc * 256 + (s + 1) * 128]),
                                (xt[dc][:, cols]),
                                start=(dc == 0), stop=(dc == 1),
                            )
                        a_ps.append(aps)
                    for s in range(2):
                        bps = pb_pool.tile([128, 512], F32, tag="b", space="PSUM")
                        for dc in range(2):
                            nc.tensor.matmul(
                                bps[:],
                                (b_lh[:, dc * 256 + s * 128: dc * 256 + (s + 1) * 128]),
                                (st[dc][:, cols]),
                                start=(dc == 0), stop=(dc == 1),
                            )
                        sb = p2sb.tile([128, 512], BF16, tag=f"silu{s}")
                        nc.scalar.activation(sb[:], bps[:], _AF_SILU)
                        y = p2sb.tile([128, 512], BF16, tag=f"y{s}")
                        nc.vector.tensor_mul(y[:], a_ps[s][:], sb[:])
                        y_sb.append(y)
                    for oc in range(2):
                        ops = po_pool.tile([128, 512], F32, tag="o", space="PSUM")
                        for s in range(2):
                            nc.tensor.matmul(
                                ops[:],
                                (o_lh[:, s * 256 + oc * 128: s * 256 + oc * 128 + 128]),
                                (y_sb[s][:]),
                                start=(s == 0), stop=False,
                            )
                        for dc in range(2):
                            nc.tensor.matmul(
                                ops[:],
                                (x_lh[:, dc * 256 + oc * 128: dc * 256 + oc * 128 + 128]),
                                (xt[dc][:, cols]),
                                start=False, stop=(dc == 1),
                            )
                        o_sb = osb_pool.tile([128, 512], F32, tag=f"o{oc}")
                        if oc == 0:
                            nc.scalar.copy(o_sb[:], ops[:])
                        else:
                            nc.vector.tensor_copy(o_sb[:], ops[:])
                        nc.sync.dma_start(
                            out_d[oc * 128:(oc + 1) * 128, n * 512:(n + 1) * 512],
                            o_sb[:])


def host_prep(inputs):
    """Host-side weight/input marshalling (layouts + static weight folds)."""
    import ml_dtypes
    bf = ml_dtypes.bfloat16
    x = np.ascontiguousarray(np.asarray(inputs["x"], dtype=np.float32))
    shared = np.ascontiguousarray(np.asarray(inputs["shared"], dtype=np.float32))
    p0 = np.asarray(inputs["p0"], np.float32)
    p1 = np.asarray(inputs["p1"], np.float32)
    p2 = np.asarray(inputs["p2"], np.float32)
    Wo = np.asarray(inputs["proj_out_w"], np.float32)

    wa = np.ascontiguousarray(p0.reshape(ER, DIM)).astype(bf)
    wb = np.ascontiguousarray(p1.reshape(ER, DIM)).astype(bf)
    WoP2 = np.einsum("od,edr->eor", Wo, p2)
    wo = np.ascontiguousarray(WoP2.transpose(0, 2, 1).reshape(ER, DIM)).astype(bf)
    wx = np.ascontiguousarray(Wo.T).astype(bf)
    w1 = np.ascontiguousarray(np.asarray(inputs["mlp_w1"], np.float32).T / HW)
    b1v = np.asarray(inputs["mlp_b1"], np.float32).reshape(HID, 1)
    w2 = np.ascontiguousarray(np.asarray(inputs["mlp_w2"], np.float32).T)
    b2v = np.asarray(inputs["mlp_b2"], np.float32).reshape(DIM, 1)
    gw = np.ascontiguousarray(np.asarray(inputs["gate_w"], np.float32).T / HW)
    fgw = np.ascontiguousarray(np.asarray(inputs["freq_gate_w"], np.float32).T)

    Th = np.zeros((H, H), np.float32)
    for i in range(H):
        for j in (i - 1, i, i + 1):
            if 0 <= j < H:
                Th[i, j] = 1.0
    taps = np.concatenate([-Th, 9.0 * np.eye(H, dtype=np.float32) - Th, -Th],
                          axis=0).astype(bf)

    shared_w = dict(
        wa=wa, wb=wb, wo=wo, wx=wx, w1=w1, b1v=b1v, w2=w2, b2v=b2v,
        gw=gw, fgw=fgw, thw=np.ascontiguousarray(taps),
        onec=np.ones((128, 1), np.float32),
        oner=np.ones((1, 128), np.float32),
        iden=np.eye(128, dtype=np.float32),
    )

    in_maps = []
    for b in range(B):
        xb = x[b]
        xh = np.zeros((H, DIM, WPAD), np.float32)
        xh[:, :, 1:W + 1] = xb.transpose(1, 0, 2)
        m = dict(shared_w)
        m["xh"] = xh.reshape(H, DIM * WPAD).astype(bf)
        m["xd"] = xb.reshape(DIM, HW).astype(bf)
        m["sh"] = shared[b].reshape(DIM, HW).astype(bf)
        in_maps.append(m)
    return in_maps


_AF_GELU = AF.Gelu
_AF_SILU = AF.Silu
_NC_CACHE = {}


def get_nc(reps=1):
    key = ("nc", reps)
    if key not in _NC_CACHE:
        _NC_CACHE[key] = build_nc(reps=reps)
    return _NC_CACHE[key]


def kernel(**inputs) -> np.ndarray:
    nc = get_nc()
    in_maps = host_prep(inputs)
    res = run_bass_kernel_spmd(nc, in_maps, core_ids=list(range(B)))
    outs = [res.results[b]["out"].reshape(DIM, H, W) for b in range(B)]
    return np.stack(outs, axis=0)


if __name__ == "__main__":
    sys.path.insert(0, "/root/problem")
    import reference as ref

    inputs = {k: np.asarray(v) for k, v in ref.setup_inputs().items()}
    got = kernel(**inputs)
    print("out", got.shape, got.dtype)

